# revision 17
# baseline (speedup 1.0000x reference)
"""BERT-CRF NER kernel for 8 Trainium2 NeuronCores.

Pure data-parallel over batch: B=256 -> 32 batches per core.
Per core:
  1. feats = bert @ W.T + b   (PE matmul, K=768 in 6 chunks + bias row)
  2. Viterbi forward scan over T=256 (DVE), exact replication of the
     reference op order: scores = trans + ld ; max ; + feat.
     Backpointers psi via is_equal + iota dot product.
  3. Backtrace (DVE): one-hot gather per step.
  4. max_p = (1/T) / sum(exp(ld_final - max)) via ACT exp-accum.
"""

import numpy as np
from contextlib import ExitStack

import concourse.bass as bass
import concourse.bacc as bacc
import concourse.tile as tile
import concourse.mybir as mybir
from concourse.bass_utils import run_bass_kernel_spmd

B, T, H, L = 256, 256, 768, 13
START, STOP = 11, 12
NEG = -10000.0

NCORES = 8
BC = B // NCORES          # 32 batches per core
BT = T * BC               # 8192 bt-columns per core (t-major: n = t*BC + b)
HCHUNKS = H // 128        # 6
NTILES = BT // 128        # 64 bt tiles of 128
GROUPS = 8                # bt groups of 1024 (8 tiles each)
TILES_PER_GROUP = NTILES // GROUPS
F32 = mybir.dt.float32
I32 = mybir.dt.int32
LP = 12                   # packed labels: [0..10, STOP]
PACK = list(range(11)) + [STOP]


def _build_program():
    nc = bacc.Bacc("TRN2", target_bir_lowering=False, debug=False)

    # ---- DRAM I/O ----
    bertT = nc.dram_tensor("bertT", [H, BT], F32, kind="ExternalInput").ap()
    # Packed label space: to' in PACK = [0..10, STOP] (START row dropped --
    # unreachable for t>=1); from' = PACK for t>=2 (STOP/START cols never
    # win an argmax there); t=1 keeps the full 13-wide from (ld0 lives on
    # START).  All drops are exact: the dropped entries lose by ~1e4.
    # consts cols: 0:72 wtp | 72:84 biasp(row0) | 84:240 trans_t1 (12x13)
    #   | 240:384 trans_p2 (12x12) | 384:397 iota13 | 397:409 iota12
    #   | 409:422 ld0   (trans/iota/ld0 on rows < BC)
    consts = nc.dram_tensor("consts", [128, 422], F32, kind="ExternalInput").ap()
    out_maxp = nc.dram_tensor("maxp", [BC], F32, kind="ExternalOutput").ap()
    out_path = nc.dram_tensor("path", [BC, T], I32, kind="ExternalOutput").ap()

    with tile.TileContext(nc) as tc, ExitStack() as ctx:
        const = ctx.enter_context(tc.tile_pool(name="const", bufs=1))
        stage = ctx.enter_context(tc.tile_pool(name="stage", bufs=2))
        fsbp = ctx.enter_context(tc.tile_pool(name="fsbp", bufs=2))
        psum = ctx.enter_context(tc.tile_pool(name="psum", bufs=8, space="PSUM"))
        scp = ctx.enter_context(tc.tile_pool(name="scp", bufs=3))
        ldp = ctx.enter_context(tc.tile_pool(name="ldp", bufs=3))
        ohp = ctx.enter_context(tc.tile_pool(name="ohp", bufs=3))

        # ---- constants: one DMA on the SP queue ----
        consts_sb = const.tile([128, 422], F32)
        nc.sync.dma_start(out=consts_sb, in_=consts)
        wt_sb = consts_sb[:, 0:HCHUNKS * LP]
        bias_sb = consts_sb[0:1, 72:72 + LP]
        trans1_sb = consts_sb[0:BC, 84:84 + LP * L].rearrange(
            "p (a b) -> p a b", b=L
        )
        trans2_sb = consts_sb[0:BC, 240:240 + LP * LP].rearrange(
            "p (a b) -> p a b", b=LP
        )
        iota13_sb = consts_sb[0:BC, 384:384 + L]
        iota12_sb = consts_sb[0:BC, 397:397 + LP]
        ld0_sb = consts_sb[0:BC, 409:409 + L]
        ones_sb = const.tile([1, 128], F32)
        nc.vector.memset(ones_sb, 1.0)
        # everything below may assume constants are resident (keeps every
        # downstream instruction at <=1 DMA-queue sync wait)
        tc.strict_bb_all_engine_barrier()

        # feats in scan layout: [b, t*LP + l']  (packed labels)
        feats2 = const.tile([BC, T * LP], F32)
        # psi history: step t (1..T-1) at free offset (t-1)*LP, values are
        # ORIGINAL tag ids (iota carries tags, not packed indices)
        psihist = const.tile([BC, (T - 1) * LP], F32)
        # path as f32, written back-to-front
        pathf = const.tile([BC, T], F32)

        # ---- feats matmul ----
        GL = TILES_PER_GROUP * LP  # 96 feats columns per group
        fsb_all = const.tile([128, NTILES * L], F32)
        for g in range(GROUPS):
            # stage the 6 h-chunks of this 1024-bt group
            stg = []
            for c in range(HCHUNKS):
                s = stage.tile([128, 1024], F32, tag=f"stage{c}")
                nc.sync.dma_start(
                    out=s, in_=bertT[c * 128:(c + 1) * 128, g * 1024:(g + 1) * 1024]
                )
                stg.append(s)
            # one psum bank per group, never reused -> no WAR sync on psum
            ps = psum.tile([128, GL], F32, tag="ps")
            for kk in range(TILES_PER_GROUP):
                for c in range(HCHUNKS):
                    nc.tensor.matmul(
                        ps[:, kk * LP:(kk + 1) * LP],
                        stg[c][:, kk * 128:(kk + 1) * 128],
                        wt_sb[:, c * LP:(c + 1) * LP],
                        start=(c == 0),
                        stop=False,
                    )
                nc.tensor.matmul(
                    ps[:, kk * LP:(kk + 1) * LP], ones_sb, bias_sb,
                    start=False, stop=True,
                )
            nc.scalar.copy(out=fsb_all[:, g * GL:(g + 1) * GL], in_=ps)
            # rearrange into scan layout via SBUF->SBUF DMA (partition move)
            # group g covers t in [g*32, (g+1)*32); tile kk covers 4 t's;
            # partition quadrant s within a tile is t = g*32 + kk*4 + s.
            f2v = feats2.rearrange("p (t l) -> p t l", l=LP)
            fsbv = fsb_all[:, g * GL:(g + 1) * GL].rearrange(
                "p (k l) -> p k l", l=LP
            )
            for s in range(4):
                nc.scalar.dma_start(
                    out=f2v[:, g * 32 + s:(g + 1) * 32:4, :],
                    in_=fsbv[s * 32:(s + 1) * 32, :, :],
                )

        # ---- forward Viterbi scan ----
        # DVE chain: sc = trans + ld ; m = max(sc) ; ld' = m + feat.
        # GPSIMD (off-chain): psiw = (sc == m) * iota, written into an
        # 8-step batch buffer; DVE folds each batch into psihist with ONE
        # strided reduce (amortizes the cross-engine join).
        PB = 8  # psi batch size (steps)
        # --- t = 1: full 13-wide 'from' (ld0 lives on START) ---
        sc1 = scp.tile([BC, LP, L], F32, tag="sc1")
        nc.vector.tensor_add(
            sc1, trans1_sb, ld0_sb[:, None, :].broadcast_to([BC, LP, L])
        )
        m1 = ldp.tile([BC, LP], F32, tag="m")
        nc.vector.reduce_max(m1, sc1, axis=mybir.AxisListType.X)
        ld_prev = ldp.tile([BC, LP], F32, tag="ld")
        nc.vector.tensor_add(ld_prev, m1, feats2[:, LP:2 * LP])
        mask1 = scp.tile([BC, LP, L], F32, tag="mask1")
        nc.vector.tensor_tensor(
            mask1, sc1, m1[:, :, None].broadcast_to([BC, LP, L]),
            op=mybir.AluOpType.is_equal,
        )
        psiw1 = scp.tile([BC, LP, L], F32, tag="psiw1")
        nc.gpsimd.tensor_mul(
            psiw1, mask1, iota13_sb[:, None, :].broadcast_to([BC, LP, L])
        )
        nc.vector.reduce_sum(psihist[:, 0:LP], psiw1, axis=mybir.AxisListType.X)
        # --- t >= 2: packed 12x12 ---
        batch = None
        bstart = None
        for t in range(2, T):
            sc = scp.tile([BC, LP, LP], F32, tag="sc")
            nc.vector.tensor_add(
                sc, trans2_sb, ld_prev[:, None, :].broadcast_to([BC, LP, LP])
            )
            m = ldp.tile([BC, LP], F32, tag="m")
            nc.vector.reduce_max(m, sc, axis=mybir.AxisListType.X)
            ld_new = ldp.tile([BC, LP], F32, tag="ld")
            nc.vector.tensor_add(
                ld_new, m, feats2[:, t * LP:(t + 1) * LP]
            )
            if batch is None:
                batch = scp.tile([BC, PB, LP, LP], F32, tag="psiwb")
                bstart = t
            j = t - bstart
            mask = scp.tile([BC, LP, LP], F32, tag="mask")
            nc.vector.tensor_tensor(
                mask, sc, m[:, :, None].broadcast_to([BC, LP, LP]),
                op=mybir.AluOpType.is_equal,
            )
            nc.gpsimd.tensor_mul(
                batch[:, j, :, :], mask,
                iota12_sb[:, None, :].broadcast_to([BC, LP, LP]),
            )
            if j == PB - 1 or t == T - 1:
                nb = j + 1
                nc.vector.reduce_sum(
                    psihist[:, (bstart - 1) * LP:(bstart - 1 + nb) * LP],
                    batch[:, 0:nb, :, :],
                    axis=mybir.AxisListType.X,
                )
                batch = None
            ld_prev = ld_new

        # ---- tail: max_p and last tag ----
        mpos = const.tile([BC, 1], F32)
        nc.vector.reduce_max(mpos, ld_prev, axis=mybir.AxisListType.X)
        negm = const.tile([BC, 1], F32)
        nc.vector.tensor_scalar_mul(negm, mpos, -1.0)
        exps = const.tile([BC, LP], F32)
        sumexp = const.tile([BC, 1], F32)
        nc.scalar.activation(
            out=exps, in_=ld_prev, func=mybir.ActivationFunctionType.Exp,
            bias=negm, scale=1.0, accum_out=sumexp,
        )
        rec = const.tile([BC, 1], F32)
        nc.vector.reciprocal(rec, sumexp)
        maxp_sb = const.tile([BC, 1], F32)
        nc.vector.tensor_scalar_mul(maxp_sb, rec, 1.0 / T)
        nc.sync.dma_start(out=out_maxp, in_=maxp_sb)

        # last = argmax(ld_final)
        maskl = const.tile([BC, LP], F32)
        nc.vector.tensor_scalar(
            maskl, ld_prev, mpos, None, op0=mybir.AluOpType.is_equal
        )
        psiwl = const.tile([BC, LP], F32)
        nc.vector.tensor_mul(psiwl, maskl, iota12_sb)
        nc.vector.reduce_sum(
            pathf[:, T - 1:T], psiwl, axis=mybir.AxisListType.X
        )

        # ---- backtrace: one fused op per step ----
        # out = (iota == path[i+1]) * psi_i ; accum = sum -> path[i]
        for i in range(T - 2, -1, -1):
            scr = ohp.tile([BC, LP], F32, tag="scr")
            nc.vector.scalar_tensor_tensor(
                out=scr,
                in0=iota12_sb,
                scalar=pathf[:, i + 1:i + 2],
                in1=psihist[:, i * LP:(i + 1) * LP],
                op0=mybir.AluOpType.is_equal,
                op1=mybir.AluOpType.mult,
                accum_out=pathf[:, i:i + 1],
            )

        # ---- emit path as int32 ----
        path_i = const.tile([BC, T], I32)
        nc.vector.tensor_copy(path_i, pathf)
        nc.sync.dma_start(out=out_path, in_=path_i)

    nc.compile()
    return nc


_NC_CACHE = None


def _get_nc():
    global _NC_CACHE
    if _NC_CACHE is None:
        _NC_CACHE = _build_program()
    return _NC_CACHE


def _make_in_maps(bert_seq_out, W, b, transitions):
    bert_seq_out = np.asarray(bert_seq_out, dtype=np.float32)
    W = np.asarray(W, dtype=np.float32)
    b = np.asarray(b, dtype=np.float32)
    transitions = np.asarray(transitions, dtype=np.float32)

    consts = np.zeros((128, 422), np.float32)
    Wp = W[PACK]                                   # [12, H]
    consts[:, 0:HCHUNKS * LP] = (
        Wp.reshape(LP, HCHUNKS, 128).transpose(2, 1, 0).reshape(128, HCHUNKS * LP)
    )
    consts[0, 72:72 + LP] = b[PACK]
    consts[:BC, 84:84 + LP * L] = transitions[PACK, :].reshape(1, LP * L)
    consts[:BC, 240:240 + LP * LP] = (
        transitions[np.ix_(PACK, PACK)].reshape(1, LP * LP)
    )
    consts[:BC, 384:384 + L] = np.arange(L, dtype=np.float32)
    consts[:BC, 397:397 + LP] = np.array(PACK, np.float32)
    consts[:BC, 409:409 + L] = NEG
    consts[:BC, 409 + START] = 0.0

    in_maps = []
    for c in range(NCORES):
        sl = bert_seq_out[c * BC:(c + 1) * BC]          # [BC, T, H]
        bertT = np.ascontiguousarray(
            sl.transpose(2, 1, 0).reshape(H, BT)         # col n = t*BC + b
        )
        in_maps.append({
            "bertT": bertT,
            "consts": consts,
        })
    return in_maps


def kernel(bert_seq_out, W, b, transitions):
    nc = _get_nc()
    in_maps = _make_in_maps(bert_seq_out, W, b, transitions)
    res = run_bass_kernel_spmd(nc, in_maps, core_ids=list(range(NCORES)))
    maxp = np.concatenate([res.results[c]["maxp"] for c in range(NCORES)], axis=0)
    path = np.concatenate([res.results[c]["path"] for c in range(NCORES)], axis=0)
    return maxp.astype(np.float32), path.astype(np.int32)


# revision 18
# speedup vs baseline: 1.0380x; 1.0380x over previous
"""BERT-CRF NER kernel for 8 Trainium2 NeuronCores.

Pure data-parallel over batch: B=256 -> 32 batches per core.
Per core:
  1. feats = bert @ W.T + b   (PE matmul, K=768 in 6 chunks + bias row)
  2. Viterbi forward scan over T=256 (DVE), exact replication of the
     reference op order: scores = trans + ld ; max ; + feat.
     Backpointers psi via is_equal + iota dot product.
  3. Backtrace (DVE): one-hot gather per step.
  4. max_p = (1/T) / sum(exp(ld_final - max)) via ACT exp-accum.
"""

import numpy as np
from contextlib import ExitStack

import concourse.bass as bass
import concourse.bacc as bacc
import concourse.tile as tile
import concourse.mybir as mybir
from concourse.bass_utils import run_bass_kernel_spmd

B, T, H, L = 256, 256, 768, 13
START, STOP = 11, 12
NEG = -10000.0

NCORES = 8
BC = B // NCORES          # 32 batches per core
BT = T * BC               # 8192 bt-columns per core (t-major: n = t*BC + b)
HCHUNKS = H // 128        # 6
NTILES = BT // 128        # 64 bt tiles of 128
GROUPS = 8                # bt groups of 1024 (8 tiles each)
TILES_PER_GROUP = NTILES // GROUPS
F32 = mybir.dt.float32
I32 = mybir.dt.int32
LP = 12                   # packed labels: [0..10, STOP]
LF = 11                   # 'from' axis for t>=2: [0..10] (STOP never wins)
PACK = list(range(11)) + [STOP]


def _build_program():
    nc = bacc.Bacc("TRN2", target_bir_lowering=False, debug=False)

    # ---- DRAM I/O ----
    bertT = nc.dram_tensor("bertT", [H, BT], F32, kind="ExternalInput").ap()
    # Packed label space: to' in PACK = [0..10, STOP] (START row dropped --
    # unreachable for t>=1); from' = PACK for t>=2 (STOP/START cols never
    # win an argmax there); t=1 keeps the full 13-wide from (ld0 lives on
    # START).  All drops are exact: the dropped entries lose by ~1e4.
    # consts cols: 0:72 wtp | 72:84 biasp(row0) | 84:240 trans_t1 (12x13)
    #   | 240:372 trans_p2 (12x11) | 384:397 iota13 | 397:409 iota12
    #   | 409:422 ld0   (trans/iota/ld0 on rows < BC)
    consts = nc.dram_tensor("consts", [128, 422], F32, kind="ExternalInput").ap()
    out_maxp = nc.dram_tensor("maxp", [BC], F32, kind="ExternalOutput").ap()
    out_path = nc.dram_tensor("path", [BC, T], I32, kind="ExternalOutput").ap()

    with tile.TileContext(nc) as tc, ExitStack() as ctx:
        const = ctx.enter_context(tc.tile_pool(name="const", bufs=1))
        stage = ctx.enter_context(tc.tile_pool(name="stage", bufs=2))
        fsbp = ctx.enter_context(tc.tile_pool(name="fsbp", bufs=2))
        psum = ctx.enter_context(tc.tile_pool(name="psum", bufs=8, space="PSUM"))
        scp = ctx.enter_context(tc.tile_pool(name="scp", bufs=3))
        ldp = ctx.enter_context(tc.tile_pool(name="ldp", bufs=3))
        ohp = ctx.enter_context(tc.tile_pool(name="ohp", bufs=3))

        # ---- constants: one DMA on the SP queue ----
        consts_sb = const.tile([128, 422], F32)
        nc.sync.dma_start(out=consts_sb, in_=consts)
        wt_sb = consts_sb[:, 0:HCHUNKS * LP]
        bias_sb = consts_sb[0:1, 72:72 + LP]
        trans1_sb = consts_sb[0:BC, 84:84 + LP * L].rearrange(
            "p (a b) -> p a b", b=L
        )
        trans2_sb = consts_sb[0:BC, 240:240 + LP * LF].rearrange(
            "p (a b) -> p a b", b=LF
        )
        iota13_sb = consts_sb[0:BC, 384:384 + L]
        iota12_sb = consts_sb[0:BC, 397:397 + LP]
        ld0_sb = consts_sb[0:BC, 409:409 + L]
        ones_sb = const.tile([1, 128], F32)
        nc.vector.memset(ones_sb, 1.0)
        # everything below may assume constants are resident (keeps every
        # downstream instruction at <=1 DMA-queue sync wait)
        tc.strict_bb_all_engine_barrier()

        # feats in scan layout: [b, t*LP + l']  (packed labels)
        feats2 = const.tile([BC, T * LP], F32)
        # psi history: step t (1..T-1) at free offset (t-1)*LP, values are
        # ORIGINAL tag ids (iota carries tags, not packed indices)
        psihist = const.tile([BC, (T - 1) * LP], F32)
        # path as f32, written back-to-front
        pathf = const.tile([BC, T], F32)

        # ---- feats matmul ----
        GL = TILES_PER_GROUP * LP  # 96 feats columns per group
        fsb_all = const.tile([128, NTILES * L], F32)
        for g in range(GROUPS):
            # stage the 6 h-chunks of this 1024-bt group
            stg = []
            for c in range(HCHUNKS):
                s = stage.tile([128, 1024], F32, tag=f"stage{c}")
                nc.sync.dma_start(
                    out=s, in_=bertT[c * 128:(c + 1) * 128, g * 1024:(g + 1) * 1024]
                )
                stg.append(s)
            # one psum bank per group, never reused -> no WAR sync on psum
            ps = psum.tile([128, GL], F32, tag="ps")
            for kk in range(TILES_PER_GROUP):
                for c in range(HCHUNKS):
                    nc.tensor.matmul(
                        ps[:, kk * LP:(kk + 1) * LP],
                        stg[c][:, kk * 128:(kk + 1) * 128],
                        wt_sb[:, c * LP:(c + 1) * LP],
                        start=(c == 0),
                        stop=False,
                    )
                nc.tensor.matmul(
                    ps[:, kk * LP:(kk + 1) * LP], ones_sb, bias_sb,
                    start=False, stop=True,
                )
            nc.scalar.copy(out=fsb_all[:, g * GL:(g + 1) * GL], in_=ps)
            # rearrange into scan layout via SBUF->SBUF DMA (partition move)
            # group g covers t in [g*32, (g+1)*32); tile kk covers 4 t's;
            # partition quadrant s within a tile is t = g*32 + kk*4 + s.
            f2v = feats2.rearrange("p (t l) -> p t l", l=LP)
            fsbv = fsb_all[:, g * GL:(g + 1) * GL].rearrange(
                "p (k l) -> p k l", l=LP
            )
            for s in range(4):
                nc.scalar.dma_start(
                    out=f2v[:, g * 32 + s:(g + 1) * 32:4, :],
                    in_=fsbv[s * 32:(s + 1) * 32, :, :],
                )

        # ---- forward Viterbi scan ----
        # DVE chain: sc = trans + ld ; m = max(sc) ; ld' = m + feat.
        # GPSIMD (off-chain): psiw = (sc == m) * iota, written into an
        # 8-step batch buffer; DVE folds each batch into psihist with ONE
        # strided reduce (amortizes the cross-engine join).
        PB = 8  # psi batch size (steps)
        # --- t = 1: full 13-wide 'from' (ld0 lives on START) ---
        sc1 = scp.tile([BC, LP, L], F32, tag="sc1")
        nc.vector.tensor_add(
            sc1, trans1_sb, ld0_sb[:, None, :].broadcast_to([BC, LP, L])
        )
        m1 = ldp.tile([BC, LP], F32, tag="m")
        nc.vector.reduce_max(m1, sc1, axis=mybir.AxisListType.X)
        ld_prev = ldp.tile([BC, LP], F32, tag="ld")
        nc.vector.tensor_add(ld_prev, m1, feats2[:, LP:2 * LP])
        mask1 = scp.tile([BC, LP, L], F32, tag="mask1")
        nc.vector.tensor_tensor(
            mask1, sc1, m1[:, :, None].broadcast_to([BC, LP, L]),
            op=mybir.AluOpType.is_equal,
        )
        psiw1 = scp.tile([BC, LP, L], F32, tag="psiw1")
        nc.gpsimd.tensor_mul(
            psiw1, mask1, iota13_sb[:, None, :].broadcast_to([BC, LP, L])
        )
        nc.vector.reduce_sum(psihist[:, 0:LP], psiw1, axis=mybir.AxisListType.X)
        # --- t >= 2: packed 12x12 ---
        batch = None
        bstart = None
        for t in range(2, T):
            sc = scp.tile([BC, LP, LF], F32, tag="sc")
            nc.vector.tensor_add(
                sc, trans2_sb,
                ld_prev[:, None, 0:LF].broadcast_to([BC, LP, LF])
            )
            m = ldp.tile([BC, LP], F32, tag="m")
            nc.vector.reduce_max(m, sc, axis=mybir.AxisListType.X)
            ld_new = ldp.tile([BC, LP], F32, tag="ld")
            nc.vector.tensor_add(
                ld_new, m, feats2[:, t * LP:(t + 1) * LP]
            )
            if batch is None:
                batch = scp.tile([BC, PB, LP, LF], F32, tag="psiwb")
                bstart = t
            j = t - bstart
            mask = scp.tile([BC, LP, LF], F32, tag="mask")
            nc.vector.tensor_tensor(
                mask, sc, m[:, :, None].broadcast_to([BC, LP, LF]),
                op=mybir.AluOpType.is_equal,
            )
            nc.gpsimd.tensor_mul(
                batch[:, j, :, :], mask,
                iota12_sb[:, None, 0:LF].broadcast_to([BC, LP, LF]),
            )
            if j == PB - 1 or t == T - 1:
                nb = j + 1
                nc.vector.reduce_sum(
                    psihist[:, (bstart - 1) * LP:(bstart - 1 + nb) * LP],
                    batch[:, 0:nb, :, :],
                    axis=mybir.AxisListType.X,
                )
                batch = None
            ld_prev = ld_new

        # ---- tail: max_p and last tag ----
        mpos = const.tile([BC, 1], F32)
        nc.vector.reduce_max(mpos, ld_prev, axis=mybir.AxisListType.X)
        negm = const.tile([BC, 1], F32)
        nc.vector.tensor_scalar_mul(negm, mpos, -1.0)
        exps = const.tile([BC, LP], F32)
        sumexp = const.tile([BC, 1], F32)
        nc.scalar.activation(
            out=exps, in_=ld_prev, func=mybir.ActivationFunctionType.Exp,
            bias=negm, scale=1.0, accum_out=sumexp,
        )
        rec = const.tile([BC, 1], F32)
        nc.vector.reciprocal(rec, sumexp)
        maxp_sb = const.tile([BC, 1], F32)
        nc.vector.tensor_scalar_mul(maxp_sb, rec, 1.0 / T)
        nc.sync.dma_start(out=out_maxp, in_=maxp_sb)

        # last = argmax(ld_final)
        maskl = const.tile([BC, LP], F32)
        nc.vector.tensor_scalar(
            maskl, ld_prev, mpos, None, op0=mybir.AluOpType.is_equal
        )
        psiwl = const.tile([BC, LP], F32)
        nc.vector.tensor_mul(psiwl, maskl, iota12_sb)
        nc.vector.reduce_sum(
            pathf[:, T - 1:T], psiwl, axis=mybir.AxisListType.X
        )

        # ---- backtrace: one fused op per step ----
        # out = (iota == path[i+1]) * psi_i ; accum = sum -> path[i]
        for i in range(T - 2, -1, -1):
            scr = ohp.tile([BC, LP], F32, tag="scr")
            nc.vector.scalar_tensor_tensor(
                out=scr,
                in0=iota12_sb,
                scalar=pathf[:, i + 1:i + 2],
                in1=psihist[:, i * LP:(i + 1) * LP],
                op0=mybir.AluOpType.is_equal,
                op1=mybir.AluOpType.mult,
                accum_out=pathf[:, i:i + 1],
            )

        # ---- emit path as int32 ----
        path_i = const.tile([BC, T], I32)
        nc.vector.tensor_copy(path_i, pathf)
        nc.sync.dma_start(out=out_path, in_=path_i)

    nc.compile()
    return nc


_NC_CACHE = None


def _get_nc():
    global _NC_CACHE
    if _NC_CACHE is None:
        _NC_CACHE = _build_program()
    return _NC_CACHE


def _make_in_maps(bert_seq_out, W, b, transitions):
    bert_seq_out = np.asarray(bert_seq_out, dtype=np.float32)
    W = np.asarray(W, dtype=np.float32)
    b = np.asarray(b, dtype=np.float32)
    transitions = np.asarray(transitions, dtype=np.float32)

    consts = np.zeros((128, 422), np.float32)
    Wp = W[PACK]                                   # [12, H]
    consts[:, 0:HCHUNKS * LP] = (
        Wp.reshape(LP, HCHUNKS, 128).transpose(2, 1, 0).reshape(128, HCHUNKS * LP)
    )
    consts[0, 72:72 + LP] = b[PACK]
    consts[:BC, 84:84 + LP * L] = transitions[PACK, :].reshape(1, LP * L)
    consts[:BC, 240:240 + LP * LF] = (
        transitions[np.ix_(PACK, PACK[:LF])].reshape(1, LP * LF)
    )
    consts[:BC, 384:384 + L] = np.arange(L, dtype=np.float32)
    consts[:BC, 397:397 + LP] = np.array(PACK, np.float32)
    consts[:BC, 409:409 + L] = NEG
    consts[:BC, 409 + START] = 0.0

    in_maps = []
    for c in range(NCORES):
        sl = bert_seq_out[c * BC:(c + 1) * BC]          # [BC, T, H]
        bertT = np.ascontiguousarray(
            sl.transpose(2, 1, 0).reshape(H, BT)         # col n = t*BC + b
        )
        in_maps.append({
            "bertT": bertT,
            "consts": consts,
        })
    return in_maps


def kernel(bert_seq_out, W, b, transitions):
    nc = _get_nc()
    in_maps = _make_in_maps(bert_seq_out, W, b, transitions)
    res = run_bass_kernel_spmd(nc, in_maps, core_ids=list(range(NCORES)))
    maxp = np.concatenate([res.results[c]["maxp"] for c in range(NCORES)], axis=0)
    path = np.concatenate([res.results[c]["path"] for c in range(NCORES)], axis=0)
    return maxp.astype(np.float32), path.astype(np.int32)


# revision 20
# speedup vs baseline: 1.0631x; 1.0242x over previous
"""BERT-CRF NER kernel for 8 Trainium2 NeuronCores.

Pure data-parallel over batch: B=256 -> 32 batches per core.
Per core:
  1. feats = bert @ W.T + b   (PE matmul, K=768 in 6 chunks + bias row)
  2. Viterbi forward scan over T=256 (DVE), exact replication of the
     reference op order: scores = trans + ld ; max ; + feat.
     Backpointers psi via is_equal + iota dot product.
  3. Backtrace (DVE): one-hot gather per step.
  4. max_p = (1/T) / sum(exp(ld_final - max)) via ACT exp-accum.
"""

import numpy as np
from contextlib import ExitStack

import concourse.bass as bass
import concourse.bacc as bacc
import concourse.tile as tile
import concourse.mybir as mybir
from concourse.bass_utils import run_bass_kernel_spmd

B, T, H, L = 256, 256, 768, 13
START, STOP = 11, 12
NEG = -10000.0

NCORES = 8
BC = B // NCORES          # 32 batches per core
BT = T * BC               # 8192 bt-columns per core (t-major: n = t*BC + b)
HCHUNKS = H // 128        # 6
NTILES = BT // 128        # 64 bt tiles of 128
GROUPS = 16               # bt groups of 512 (4 tiles each)
TILES_PER_GROUP = NTILES // GROUPS
F32 = mybir.dt.float32
I32 = mybir.dt.int32
LP = 12                   # packed labels: [0..10, STOP]
LF = 11                   # 'from' axis for t>=2: [0..10] (STOP never wins)
PACK = list(range(11)) + [STOP]


def _build_program():
    nc = bacc.Bacc("TRN2", target_bir_lowering=False, debug=False)

    # ---- DRAM I/O ----
    bertT = nc.dram_tensor("bertT", [H, BT], F32, kind="ExternalInput").ap()
    # Packed label space: to' in PACK = [0..10, STOP] (START row dropped --
    # unreachable for t>=1); from' = PACK for t>=2 (STOP/START cols never
    # win an argmax there); t=1 keeps the full 13-wide from (ld0 lives on
    # START).  All drops are exact: the dropped entries lose by ~1e4.
    # consts cols: 0:72 wtp | 72:84 biasp(row0) | 84:240 trans_t1 (12x13)
    #   | 240:372 trans_p2 (12x11) | 384:397 iota13 | 397:409 iota12
    #   | 409:422 ld0   (trans/iota/ld0 on rows < BC)
    consts = nc.dram_tensor("consts", [128, 422], F32, kind="ExternalInput").ap()
    out_maxp = nc.dram_tensor("maxp", [BC], F32, kind="ExternalOutput").ap()
    out_path = nc.dram_tensor("path", [BC, T], I32, kind="ExternalOutput").ap()

    with tile.TileContext(nc) as tc, ExitStack() as ctx:
        const = ctx.enter_context(tc.tile_pool(name="const", bufs=1))
        stage = ctx.enter_context(tc.tile_pool(name="stage", bufs=2))
        fsbp = ctx.enter_context(tc.tile_pool(name="fsbp", bufs=2))
        psum = ctx.enter_context(tc.tile_pool(name="psum", bufs=8, space="PSUM"))
        scp = ctx.enter_context(tc.tile_pool(name="scp", bufs=3))
        ldp = ctx.enter_context(tc.tile_pool(name="ldp", bufs=3))
        ohp = ctx.enter_context(tc.tile_pool(name="ohp", bufs=3))

        # ---- constants: one DMA on the SP queue ----
        consts_sb = const.tile([128, 422], F32)
        nc.sync.dma_start(out=consts_sb, in_=consts)
        wt_sb = consts_sb[:, 0:HCHUNKS * LP]
        bias_sb = consts_sb[0:1, 72:72 + LP]
        trans1_sb = consts_sb[0:BC, 84:84 + LP * L].rearrange(
            "p (a b) -> p a b", b=L
        )
        trans2_sb = consts_sb[0:BC, 240:240 + LP * LF].rearrange(
            "p (a b) -> p a b", b=LF
        )
        iota13_sb = consts_sb[0:BC, 384:384 + L]
        iota12_sb = consts_sb[0:BC, 397:397 + LP]
        ld0_sb = consts_sb[0:BC, 409:409 + L]
        ones_sb = const.tile([1, 128], F32)
        nc.vector.memset(ones_sb, 1.0)
        # everything below may assume constants are resident (keeps every
        # downstream instruction at <=1 DMA-queue sync wait)
        tc.strict_bb_all_engine_barrier()

        # feats in scan layout: [b, t*LP + l']  (packed labels)
        feats2 = const.tile([BC, T * LP], F32)
        # psi history: step t (1..T-1) at free offset (t-1)*LP, values are
        # ORIGINAL tag ids (iota carries tags, not packed indices)
        psihist = const.tile([BC, (T - 1) * LP], F32)
        # path as f32, written back-to-front
        pathf = const.tile([BC, T], F32)

        # ---- feats matmul ----
        GL = TILES_PER_GROUP * LP  # 96 feats columns per group
        fsb_all = const.tile([128, NTILES * L], F32)
        for g in range(GROUPS):
            # stage the 6 h-chunks of this 1024-bt group
            stg = []
            for c in range(HCHUNKS):
                s = stage.tile([128, BT // GROUPS], F32, tag=f"stage{c}")
                nc.sync.dma_start(
                    out=s,
                    in_=bertT[c * 128:(c + 1) * 128,
                              g * (BT // GROUPS):(g + 1) * (BT // GROUPS)],
                )
                stg.append(s)
            # one psum bank per group, never reused -> no WAR sync on psum
            ps = psum.tile([128, GL], F32, tag="ps")
            for kk in range(TILES_PER_GROUP):
                for c in range(HCHUNKS):
                    nc.tensor.matmul(
                        ps[:, kk * LP:(kk + 1) * LP],
                        stg[c][:, kk * 128:(kk + 1) * 128],
                        wt_sb[:, c * LP:(c + 1) * LP],
                        start=(c == 0),
                        stop=False,
                    )
                nc.tensor.matmul(
                    ps[:, kk * LP:(kk + 1) * LP], ones_sb, bias_sb,
                    start=False, stop=True,
                )
            nc.scalar.copy(out=fsb_all[:, g * GL:(g + 1) * GL], in_=ps)
            # rearrange into scan layout via SBUF->SBUF DMA (partition move)
            # group g covers t in [g*32, (g+1)*32); tile kk covers 4 t's;
            # partition quadrant s within a tile is t = g*32 + kk*4 + s.
            f2v = feats2.rearrange("p (t l) -> p t l", l=LP)
            fsbv = fsb_all[:, g * GL:(g + 1) * GL].rearrange(
                "p (k l) -> p k l", l=LP
            )
            TSPAN = T // GROUPS
            for s in range(4):
                nc.scalar.dma_start(
                    out=f2v[:, g * TSPAN + s:(g + 1) * TSPAN:4, :],
                    in_=fsbv[s * 32:(s + 1) * 32, :, :],
                )

        # ---- forward Viterbi scan ----
        # DVE chain: sc = trans + ld ; m = max(sc) ; ld' = m + feat.
        # GPSIMD (off-chain): psiw = (sc == m) * iota, written into an
        # 8-step batch buffer; DVE folds each batch into psihist with ONE
        # strided reduce (amortizes the cross-engine join).
        PB = 8  # psi batch size (steps)
        # --- t = 1: full 13-wide 'from' (ld0 lives on START) ---
        sc1 = scp.tile([BC, LP, L], F32, tag="sc1")
        nc.vector.tensor_add(
            sc1, trans1_sb, ld0_sb[:, None, :].broadcast_to([BC, LP, L])
        )
        m1 = ldp.tile([BC, LP], F32, tag="m")
        nc.vector.reduce_max(m1, sc1, axis=mybir.AxisListType.X)
        ld_prev = ldp.tile([BC, LP], F32, tag="ld")
        nc.vector.tensor_add(ld_prev, m1, feats2[:, LP:2 * LP])
        mask1 = scp.tile([BC, LP, L], F32, tag="mask1")
        nc.vector.tensor_tensor(
            mask1, sc1, m1[:, :, None].broadcast_to([BC, LP, L]),
            op=mybir.AluOpType.is_equal,
        )
        psiw1 = scp.tile([BC, LP, L], F32, tag="psiw1")
        nc.gpsimd.tensor_mul(
            psiw1, mask1, iota13_sb[:, None, :].broadcast_to([BC, LP, L])
        )
        nc.vector.reduce_sum(psihist[:, 0:LP], psiw1, axis=mybir.AxisListType.X)
        # --- t >= 2: packed 12x12 ---
        batch = None
        bstart = None
        for t in range(2, T):
            sc = scp.tile([BC, LP, LF], F32, tag="sc")
            nc.vector.tensor_add(
                sc, trans2_sb,
                ld_prev[:, None, 0:LF].broadcast_to([BC, LP, LF])
            )
            m = ldp.tile([BC, LP], F32, tag="m")
            nc.vector.reduce_max(m, sc, axis=mybir.AxisListType.X)
            ld_new = ldp.tile([BC, LP], F32, tag="ld")
            nc.vector.tensor_add(
                ld_new, m, feats2[:, t * LP:(t + 1) * LP]
            )
            if batch is None:
                batch = scp.tile([BC, PB, LP, LF], F32, tag="psiwb")
                bstart = t
            j = t - bstart
            mask = scp.tile([BC, LP, LF], F32, tag="mask")
            nc.vector.tensor_tensor(
                mask, sc, m[:, :, None].broadcast_to([BC, LP, LF]),
                op=mybir.AluOpType.is_equal,
            )
            nc.gpsimd.tensor_mul(
                batch[:, j, :, :], mask,
                iota12_sb[:, None, 0:LF].broadcast_to([BC, LP, LF]),
            )
            if j == PB - 1 or t == T - 1:
                nb = j + 1
                nc.vector.reduce_sum(
                    psihist[:, (bstart - 1) * LP:(bstart - 1 + nb) * LP],
                    batch[:, 0:nb, :, :],
                    axis=mybir.AxisListType.X,
                )
                batch = None
            ld_prev = ld_new

        # ---- tail: max_p and last tag ----
        mpos = const.tile([BC, 1], F32)
        nc.vector.reduce_max(mpos, ld_prev, axis=mybir.AxisListType.X)
        negm = const.tile([BC, 1], F32)
        nc.vector.tensor_scalar_mul(negm, mpos, -1.0)
        exps = const.tile([BC, LP], F32)
        sumexp = const.tile([BC, 1], F32)
        nc.scalar.activation(
            out=exps, in_=ld_prev, func=mybir.ActivationFunctionType.Exp,
            bias=negm, scale=1.0, accum_out=sumexp,
        )
        rec = const.tile([BC, 1], F32)
        nc.vector.reciprocal(rec, sumexp)
        maxp_sb = const.tile([BC, 1], F32)
        nc.vector.tensor_scalar_mul(maxp_sb, rec, 1.0 / T)
        nc.sync.dma_start(out=out_maxp, in_=maxp_sb)

        # last = argmax(ld_final)
        maskl = const.tile([BC, LP], F32)
        nc.vector.tensor_scalar(
            maskl, ld_prev, mpos, None, op0=mybir.AluOpType.is_equal
        )
        psiwl = const.tile([BC, LP], F32)
        nc.vector.tensor_mul(psiwl, maskl, iota12_sb)
        nc.vector.reduce_sum(
            pathf[:, T - 1:T], psiwl, axis=mybir.AxisListType.X
        )

        # ---- backtrace: one fused op per step ----
        # out = (iota == path[i+1]) * psi_i ; accum = sum -> path[i]
        for i in range(T - 2, -1, -1):
            scr = ohp.tile([BC, LP], F32, tag="scr")
            nc.vector.scalar_tensor_tensor(
                out=scr,
                in0=iota12_sb,
                scalar=pathf[:, i + 1:i + 2],
                in1=psihist[:, i * LP:(i + 1) * LP],
                op0=mybir.AluOpType.is_equal,
                op1=mybir.AluOpType.mult,
                accum_out=pathf[:, i:i + 1],
            )

        # ---- emit path as int32 ----
        path_i = const.tile([BC, T], I32)
        nc.vector.tensor_copy(path_i, pathf)
        nc.sync.dma_start(out=out_path, in_=path_i)

    nc.compile()
    return nc


_NC_CACHE = None


def _get_nc():
    global _NC_CACHE
    if _NC_CACHE is None:
        _NC_CACHE = _build_program()
    return _NC_CACHE


def _make_in_maps(bert_seq_out, W, b, transitions):
    bert_seq_out = np.asarray(bert_seq_out, dtype=np.float32)
    W = np.asarray(W, dtype=np.float32)
    b = np.asarray(b, dtype=np.float32)
    transitions = np.asarray(transitions, dtype=np.float32)

    consts = np.zeros((128, 422), np.float32)
    Wp = W[PACK]                                   # [12, H]
    consts[:, 0:HCHUNKS * LP] = (
        Wp.reshape(LP, HCHUNKS, 128).transpose(2, 1, 0).reshape(128, HCHUNKS * LP)
    )
    consts[0, 72:72 + LP] = b[PACK]
    consts[:BC, 84:84 + LP * L] = transitions[PACK, :].reshape(1, LP * L)
    consts[:BC, 240:240 + LP * LF] = (
        transitions[np.ix_(PACK, PACK[:LF])].reshape(1, LP * LF)
    )
    consts[:BC, 384:384 + L] = np.arange(L, dtype=np.float32)
    consts[:BC, 397:397 + LP] = np.array(PACK, np.float32)
    consts[:BC, 409:409 + L] = NEG
    consts[:BC, 409 + START] = 0.0

    in_maps = []
    for c in range(NCORES):
        sl = bert_seq_out[c * BC:(c + 1) * BC]          # [BC, T, H]
        bertT = np.ascontiguousarray(
            sl.transpose(2, 1, 0).reshape(H, BT)         # col n = t*BC + b
        )
        in_maps.append({
            "bertT": bertT,
            "consts": consts,
        })
    return in_maps


def kernel(bert_seq_out, W, b, transitions):
    nc = _get_nc()
    in_maps = _make_in_maps(bert_seq_out, W, b, transitions)
    res = run_bass_kernel_spmd(nc, in_maps, core_ids=list(range(NCORES)))
    maxp = np.concatenate([res.results[c]["maxp"] for c in range(NCORES)], axis=0)
    path = np.concatenate([res.results[c]["path"] for c in range(NCORES)], axis=0)
    return maxp.astype(np.float32), path.astype(np.int32)
